# revision 8
# baseline (speedup 1.0000x reference)
"""BloomAttention (B=2, S=1024, H=4096, 32 heads, head_dim=128) on 8 TRN2
NeuronCores — tensor-parallel over heads (4 heads per core).

v3: bf16 data path + LDWEIGHTS minimization. Loops are ordered so that
consecutive matmuls share the same stationary operand (ht-major QKV with
8 concurrent PSUM accumulation chains, f-major projection, j-major
attention with the denominator chains deferred so all ones-matmuls are
back-to-back), and a post-scheduling pass deletes consecutive identical
InstLdweights — the paired InstMatmult (ldweights=False) then reuses the
weights already resident in the PE array.

Per core, SPMD:
  * hidT [B, H, S] bf16 (replicated), w_qkv sliced per-head column-wise,
    w_out row-wise; each core computes attention for its 4 heads:
      qT/kT = w.T @ hidT (feature-major), vN = hidT.T @ wv (seq-major)
      scoresT[k,q] = kT.T @ qT; P = exp(scoresT)*EAD (alibi+causal baked
      into the EAD table as exact zeros)
      ctxT = (vN.T @ P) * recip(1s.T @ P)
      outT += wo.T @ ctxT
  * Each core writes its partial outT [H, B*S] in bf16; the host sums the
    8 partials and adds b_out + b_v @ w_out (v-bias commutes through the
    softmax-linear ops). q/k biases + attention scaling applied on-chip
    during PSUM eviction. Attention diagonal blocks causally trimmed at
    128-query granularity.
"""

import math
import numpy as np
from contextlib import ExitStack

import concourse.bass as bass
import concourse.tile as tile
import concourse.mybir as mybir
from concourse import bacc
from concourse.bass_utils import run_bass_kernel_spmd

f32 = mybir.dt.float32
bf16 = mybir.dt.bfloat16
AF = mybir.ActivationFunctionType
ALU = mybir.AluOpType

B, S, H = 2, 1024, 4096
TOTAL_HEADS = 32
N_CORES = 8
HPC = TOTAL_HEADS // N_CORES      # heads per core
HD = HPC * 128                    # per-core head feature width
OFF = 384                         # D-table offset
W = OFF + S                       # D-table width
MASK_FILL = -1.0e5
N_CHUNKS = 4
VERSION = 3  # bumped on every kernel change: keys the NEFF cache via the
             # nonce tensor shape (the HLO hash only sees I/O signatures)


def _dedupe_ldweights(nc):
    """Remove InstLdweights that reload the exact weights already resident
    in the PE array (same tile/offset/pattern, no intervening PE op that
    could clobber them). Any sync on the removed LDW is carried onto the
    next matmul. Runs after tile scheduling, before bacc compile."""
    n_removed = 0
    for fn in nc.m.functions:
        for blk in fn.blocks:
            new_insts = []
            last_sig = None
            pending = []
            for inst in blk.instructions:
                tn = type(inst).__name__
                if tn == "InstLdweights":
                    ap = inst.ins[0]
                    sig = (ap.memref, ap.offset, str(ap.ap), str(ap.dtype),
                           str(inst.is_transpose), str(inst.tile_position),
                           str(inst.perf_mode))
                    if sig == last_sig:
                        si = inst.sync_info
                        if si is not None and (si.on_wait or si.on_update):
                            pending.append(si)
                        n_removed += 1
                        continue
                    last_sig = sig
                    new_insts.append(inst)
                elif tn == "InstMatmult":
                    if pending:
                        si = inst.sync_info
                        if si is None:
                            si = mybir.SyncInfo(on_wait=[], on_update=[])
                            inst.sync_info = si
                        for p in pending:
                            si.on_wait.extend(p.on_wait)
                            si.on_update.extend(p.on_update)
                        pending = []
                    new_insts.append(inst)
                else:
                    if (getattr(inst, "engine", None) == mybir.EngineType.PE
                            and tn != "InstEventSemaphore"):
                        last_sig = None
                    new_insts.append(inst)
            assert not pending
            blk.instructions[:] = new_insts
    return n_removed


def _build_nc(n_devices=N_CORES, repeat=1, nonce=1):
    hpc = HPC
    NH_T = H // 128               # 32 k-tiles
    CH = NH_T // N_CHUNKS         # 8 k-tiles per chunk
    SB = S // 512                 # 2 query blocks
    ST = S // 128                 # 8 seq tiles
    scaling = float(128 ** -0.5)

    nc = bacc.Bacc("TRN2", target_bir_lowering=False, debug=False,
                   num_devices=n_devices)
    hidT = nc.dram_tensor("hidT", [B, H, S], bf16, kind="ExternalInput").ap()
    wq = nc.dram_tensor("wq", [H, HD], bf16, kind="ExternalInput").ap()
    wk = nc.dram_tensor("wk", [H, HD], bf16, kind="ExternalInput").ap()
    wv = nc.dram_tensor("wv", [H, HD], bf16, kind="ExternalInput").ap()
    wo = nc.dram_tensor("wo", [HD, H], bf16, kind="ExternalInput").ap()
    bq = nc.dram_tensor("bq", [128, hpc], f32, kind="ExternalInput").ap()
    bk = nc.dram_tensor("bk", [128, hpc], f32, kind="ExternalInput").ap()
    slp = nc.dram_tensor("slp", [128, hpc], f32, kind="ExternalInput").ap()
    outp = nc.dram_tensor("outp", [H, B * S], bf16, kind="ExternalOutput").ap()
    nonce_t = nc.dram_tensor("nonce", [VERSION, int(nonce)], f32,
                             kind="ExternalInput").ap()
    del nonce_t

    with tile.TileContext(nc) as tc:
        with ExitStack() as ctx:
            const = ctx.enter_context(tc.tile_pool(name="const", bufs=1))
            hidp = ctx.enter_context(tc.tile_pool(name="hidp", bufs=22))
            wsp = ctx.enter_context(tc.tile_pool(name="wsp", bufs=30))
            qkp = ctx.enter_context(tc.tile_pool(name="qkp", bufs=18))
            vnp = ctx.enter_context(tc.tile_pool(name="vnp", bufs=18))
            pp = ctx.enter_context(tc.tile_pool(name="pp", bufs=14))
            rp = ctx.enter_context(tc.tile_pool(name="rp", bufs=2))
            ctxp = ctx.enter_context(tc.tile_pool(name="ctxp", bufs=18))
            wop = ctx.enter_context(tc.tile_pool(name="wop", bufs=8))
            osp = ctx.enter_context(tc.tile_pool(name="osp", bufs=4))
            psp = ctx.enter_context(tc.tile_pool(name="psp", bufs=8, space="PSUM"))

            ps_ctr = [0]

            def ps_tile(cols=512):
                ps_ctr[0] += 1
                return psp.tile([128, cols], f32, tag="ps",
                                name=f"ps_{ps_ctr[0]}")

            # ---- constants (once) ----
            Dext = const.tile([128, W], f32, tag="dext")
            nc.gpsimd.iota(Dext[:], base=OFF, channel_multiplier=1,
                           pattern=[[-1, W]],
                           allow_small_or_imprecise_dtypes=True)
            nc.gpsimd.affine_select(Dext[:], Dext[:], base=-OFF,
                                    channel_multiplier=-1, pattern=[[1, W]],
                                    compare_op=ALU.is_ge, fill=MASK_FILL)
            ones_f = const.tile([128, 128], f32, tag="onesf")
            nc.gpsimd.memset(ones_f[:], 1.0)
            ones = const.tile([128, 128], bf16, tag="ones")
            nc.vector.tensor_copy(ones[:], ones_f[:])
            bq_t = const.tile([128, hpc], f32, tag="bq")
            nc.sync.dma_start(bq_t[:], bq[:])
            bk_t = const.tile([128, hpc], f32, tag="bk")
            nc.sync.dma_start(bk_t[:], bk[:])
            slp_t = const.tile([128, hpc], f32, tag="slp")
            nc.sync.dma_start(slp_t[:], slp[:])
            # EAD = exp(slope * (j - i)) with causal mask as exact zeros;
            # shared across batches and repeats.
            EAD = []
            for head in range(hpc):
                ead = const.tile([128, W], bf16, tag=f"ead{head}")
                nc.scalar.activation(ead[:], Dext[:], AF.Exp,
                                     scale=slp_t[:, head:head + 1])
                EAD.append(ead)

            for r in range(repeat):
                # ================= QKV (both batches per chunk) ==========
                qk_final = {
                    (b, w): [qkp.tile([128, S], bf16, tag="qkT",
                                      name=f"{w}T_{r}_{b}_{i}")
                             for i in range(hpc)]
                    for b in range(B) for w in ("q", "k")
                }
                v_final = {
                    b: [vnp.tile([128, HD], bf16, tag="vn",
                                 name=f"vN_{r}_{b}_{i}") for i in range(ST)]
                    for b in range(B)
                }

                for hc in range(N_CHUNKS):
                    hts = list(range(hc * CH, (hc + 1) * CH))
                    hid_t = {}
                    w_t = {}
                    for ht in hts:
                        wqt = wsp.tile([128, HD], bf16, tag="w",
                                       name=f"wq_{r}_{ht}")
                        nc.sync.dma_start(wqt[:], wq[ht * 128:(ht + 1) * 128, :])
                        w_t[("q", ht)] = wqt
                        for b in range(B):
                            t = hidp.tile([128, S], bf16, tag="hidt",
                                          name=f"hid_{r}_{b}_{ht}")
                            nc.sync.dma_start(
                                t[:], hidT[b, ht * 128:(ht + 1) * 128, :])
                            hid_t[(b, ht)] = t

                    # Q then K: ht-major, 8 concurrent chains (head x sub),
                    # so the 2 sub-matmuls per (ht, head) share LDWEIGHTS.
                    for which, bias_t, sc in (("q", bq_t, scaling),
                                              ("k", bk_t, 1.0)):
                        if which == "k":
                            for ht in hts:
                                wkt = wsp.tile([128, HD], bf16, tag="w",
                                               name=f"wk_{r}_{ht}")
                                nc.sync.dma_start(
                                    wkt[:], wk[ht * 128:(ht + 1) * 128, :])
                                w_t[("k", ht)] = wkt
                        for b in range(B):
                            ps = {(head, sub): ps_tile()
                                  for head in range(hpc) for sub in range(SB)}
                            for i, ht in enumerate(hts):
                                for head in range(hpc):
                                    for sub in range(SB):
                                        nc.tensor.matmul(
                                            ps[(head, sub)][:],
                                            w_t[(which, ht)][:, head * 128:(head + 1) * 128],
                                            hid_t[(b, ht)][:, sub * 512:(sub + 1) * 512],
                                            start=(i == 0), stop=(i == CH - 1))
                            for head in range(hpc):
                                for sub in range(SB):
                                    fin = qk_final[(b, which)][head]
                                    dst = fin[:, sub * 512:(sub + 1) * 512]
                                    if hc == 0:
                                        nc.scalar.activation(
                                            dst, ps[(head, sub)][:], AF.Identity,
                                            bias=bias_t[:, head:head + 1],
                                            scale=sc)
                                    else:
                                        nc.vector.scalar_tensor_tensor(
                                            out=dst, in0=ps[(head, sub)][:],
                                            scalar=sc, in1=dst,
                                            op0=ALU.mult, op1=ALU.add)

                    # V: ht-major, 8 concurrent chains (one per s-tile)
                    for ht in hts:
                        wvt = wsp.tile([128, HD], bf16, tag="w",
                                       name=f"wv_{r}_{ht}")
                        nc.sync.dma_start(wvt[:], wv[ht * 128:(ht + 1) * 128, :])
                        w_t[("v", ht)] = wvt
                    for b in range(B):
                        ps = {st: ps_tile(HD) for st in range(ST)}
                        for i, ht in enumerate(hts):
                            for st in range(ST):
                                nc.tensor.matmul(
                                    ps[st][:],
                                    hid_t[(b, ht)][:, st * 128:(st + 1) * 128],
                                    w_t[("v", ht)][:],
                                    start=(i == 0), stop=(i == CH - 1))
                        for st in range(ST):
                            vt = v_final[b][st]
                            if hc == 0:
                                nc.scalar.copy(vt[:], ps[st][:])
                            else:
                                nc.vector.tensor_add(vt[:], ps[st][:], vt[:])

                # ================= attention =================
                # Per (b, head): j-major with both query blocks inner, so
                # scores/ctx matmuls at the same j share LDWEIGHTS; the
                # denominator ones-matmuls are deferred until all P tiles
                # exist, so they form one back-to-back ones-weighted group.
                ctx_tiles = {b: [[None] * SB for _ in range(hpc)]
                             for b in range(B)}
                for b in range(B):
                    for h in range(hpc):
                        qT = qk_final[(b, "q")][h]
                        kT = qk_final[(b, "k")][h]
                        ps_ctx = {qb: ps_tile() for qb in range(SB)}
                        ps_den = {qb: ps_tile() for qb in range(SB)}
                        P_all = []  # (qb, j, qoff, n, P)
                        for j in range(ST):
                            qbs = [qb for qb in range(SB)
                                   if j < 4 * (qb + 1)]
                            Ps = {}
                            for qb in qbs:
                                qoff = max(0, 128 * j - 512 * qb)
                                n = 512 - qoff
                                ps_s = ps_tile()
                                nc.tensor.matmul(
                                    ps_s[:, :n],
                                    kT[:, j * 128:(j + 1) * 128],
                                    qT[:, qb * 512 + qoff:(qb + 1) * 512],
                                    start=True, stop=True)
                                P = pp.tile([128, 512], bf16, tag="P")
                                nc.scalar.activation(P[:, :n], ps_s[:, :n],
                                                     AF.Exp)
                                w0 = qb * 512 + qoff - j * 128 + OFF
                                nc.vector.tensor_mul(
                                    P[:, :n], P[:, :n], EAD[h][:, w0:w0 + n])
                                Ps[qb] = (qoff, n, P)
                                P_all.append((qb, j, qoff, n, P))
                            for qb in qbs:
                                qoff, n, P = Ps[qb]
                                nc.tensor.matmul(
                                    ps_ctx[qb][:, qoff:512],
                                    v_final[b][j][:, h * 128:(h + 1) * 128],
                                    P[:, :n],
                                    start=(j == 0),
                                    stop=(j == 4 * (qb + 1) - 1))
                        # denominators: one LDWEIGHTS (ones) for all 12
                        for (qb, j, qoff, n, P) in P_all:
                            nc.tensor.matmul(
                                ps_den[qb][:, qoff:512], ones[:], P[:, :n],
                                start=(j == 0), stop=(j == 4 * (qb + 1) - 1))
                        for qb in range(SB):
                            recip = rp.tile([128, 512], f32, tag="recip")
                            nc.vector.reciprocal(recip[:], ps_den[qb][:])
                            ctx_t = ctxp.tile([128, 512], bf16, tag="ctx")
                            nc.vector.tensor_mul(ctx_t[:], ps_ctx[qb][:],
                                                 recip[:])
                            ctx_tiles[b][h][qb] = ctx_t

                # ================= projection =================
                # f-major with 8 concurrent chains (ot x sb) per (og, b):
                # the 2 sb-matmuls per (f, ot) share LDWEIGHTS.
                OG = H // 512
                for og in range(OG):
                    wo_t = []
                    for f in range(hpc):
                        t = wop.tile([128, 512], bf16, tag="wo",
                                     name=f"wo_{r}_{og}_{f}")
                        nc.gpsimd.dma_start(
                            t[:],
                            wo[f * 128:(f + 1) * 128, og * 512:(og + 1) * 512])
                        wo_t.append(t)
                    for b in range(B):
                        ps = {(ot, sb): ps_tile()
                              for ot in range(4) for sb in range(SB)}
                        for f in range(hpc):
                            for ot in range(4):
                                for sb in range(SB):
                                    nc.tensor.matmul(
                                        ps[(ot, sb)][:],
                                        wo_t[f][:, ot * 128:(ot + 1) * 128],
                                        ctx_tiles[b][f][sb][:],
                                        start=(f == 0), stop=(f == hpc - 1))
                        for ot in range(4):
                            for sb in range(SB):
                                ost = osp.tile([128, 512], bf16, tag="ost")
                                nc.scalar.copy(ost[:], ps[(ot, sb)][:])
                                r0 = og * 512 + ot * 128
                                c0 = b * S + sb * 512
                                nc.gpsimd.dma_start(
                                    outp[r0:r0 + 128, c0:c0 + 512], ost[:])

    n_removed = _dedupe_ldweights(nc)
    nc.compile()
    nc._ldw_removed = n_removed
    return nc


def _alibi_slopes(total_heads):
    closest = 2 ** math.floor(math.log2(total_heads))
    base = 2 ** (-(2 ** (-(math.log2(closest) - 3))))
    powers = np.arange(1, 1 + closest, dtype=np.float32)
    slopes = np.power(base, powers).astype(np.float32)
    if closest != total_heads:
        extra_base = 2 ** (-(2 ** (-(math.log2(2 * closest) - 3))))
        num_rem = min(closest, total_heads - closest)
        extra = np.arange(1, 1 + 2 * num_rem, 2, dtype=np.float32)
        slopes = np.concatenate(
            [slopes, np.power(extra_base, extra).astype(np.float32)])
    return slopes


_NC_CACHE = {}


def _get_nc():
    if "nc" not in _NC_CACHE:
        _NC_CACHE["nc"] = _build_nc()
    return _NC_CACHE["nc"]


def _np_bf16():
    import ml_dtypes
    return np.dtype(ml_dtypes.bfloat16)


def make_in_maps(hidden_states, w_qkv, b_qkv, w_out):
    """Build the 8 per-core input dicts."""
    nbf = _np_bf16()
    scaling = np.float32(128 ** -0.5)
    hidT = np.ascontiguousarray(
        hidden_states.transpose(0, 2, 1)).astype(nbf)
    slopes = _alibi_slopes(TOTAL_HEADS)
    nonce = np.zeros((VERSION, 1), np.float32)
    in_maps = []
    for core in range(N_CORES):
        c0 = core * HD
        bq_v = b_qkv[c0:c0 + HD]
        bk_v = b_qkv[H + c0:H + c0 + HD]
        sl = slopes[core * HPC:(core + 1) * HPC]
        in_maps.append(dict(
            hidT=hidT,
            wq=np.ascontiguousarray(w_qkv[:, c0:c0 + HD]).astype(nbf),
            wk=np.ascontiguousarray(w_qkv[:, H + c0:H + c0 + HD]).astype(nbf),
            wv=np.ascontiguousarray(
                w_qkv[:, 2 * H + c0:2 * H + c0 + HD]).astype(nbf),
            wo=np.ascontiguousarray(w_out[c0:c0 + HD, :]).astype(nbf),
            bq=np.ascontiguousarray(
                (bq_v.reshape(HPC, 128).T * scaling).astype(np.float32)),
            bk=np.ascontiguousarray(bk_v.reshape(HPC, 128).T.astype(np.float32)),
            slp=np.ascontiguousarray(
                np.broadcast_to(sl[None, :], (128, HPC)).astype(np.float32)),
            nonce=nonce,
        ))
    return in_maps


def finish_output(partials, b_qkv, w_out, b_out):
    """Host-side all-reduce over cores + layout fix + bias."""
    total = np.zeros((H, B * S), dtype=np.float32)
    for p in partials:
        total += np.asarray(p).astype(np.float32)
    bias_vec = (b_qkv[2 * H:].astype(np.float64) @ w_out.astype(np.float64)
                + b_out.astype(np.float64)).astype(np.float32)
    out = total.reshape(H, B, S).transpose(1, 2, 0) + bias_vec[None, None, :]
    return np.ascontiguousarray(out.astype(np.float32))


def kernel(hidden_states, w_qkv, b_qkv, w_out, b_out):
    hidden_states = np.asarray(hidden_states, dtype=np.float32)
    w_qkv = np.asarray(w_qkv, dtype=np.float32)
    b_qkv = np.asarray(b_qkv, dtype=np.float32)
    w_out = np.asarray(w_out, dtype=np.float32)
    b_out = np.asarray(b_out, dtype=np.float32)

    nc = _get_nc()
    in_maps = make_in_maps(hidden_states, w_qkv, b_qkv, w_out)
    res = run_bass_kernel_spmd(nc, in_maps, core_ids=list(range(N_CORES)))
    return finish_output([res.results[c]["outp"] for c in range(N_CORES)],
                         b_qkv, w_out, b_out)
